# revision 10
# baseline (speedup 1.0000x reference)
"""AttentionDecoderRNN Trainium2 kernel (8 NeuronCores, SPMD).

Math (the reference's attention softmax is over a singleton dim -> weights all
ones -> ctx = features.sum(axis=1), constant over time):

    ctx   = features.sum(1)                                   (64, 1024)
    x_t   = embed[captions[:, t]]                             t = 0..30
    gates = [x_t, ctx] @ W_ih.T + b_ih + h @ W_hh.T + b_hh    (64, 4096)
    i,f,g,o -> LSTM update -> h_t                             (64, 1024)
    out[t*64+b, :] = h_t[b] @ lin_W.T + lin_b                 (1984, 32000)

Sharding: the LSTM recurrence is gate-sharded 8 ways (each core owns 128
hidden dims = 512 of the 4096 gate rows, reordered host-side to [f i o g]);
per step each core's h shard (64x128, bf16) is exchanged with an 8-rank
AllGather (Shared-addr-space output - the fast HBM-HBM path) and the
gathered (1024,64) buffer is brought back with two strided DMAs on separate
queues so the gates matmuls can start as soon as the first half lands.
The 32000-vocab output projection is column-sharded (4000 per core), runs in
bf16 on TensorE, and is paced one group per step. The x_pre precompute
(embeddings @ W_x + ctx term) runs in bf16. A dummy AllGather at kernel
start absorbs the collective entry barrier while weights stream in.
"""

import os
import sys

sys.path.insert(0, "/opt/trn_rl_repo")

import numpy as np
import ml_dtypes

import concourse.bass as bass
import concourse.tile as tile
from concourse import bacc, mybir
from concourse.bass_utils import run_bass_kernel_spmd

F32 = mybir.dt.float32
F32R = mybir.dt.float32r
BF16 = mybir.dt.bfloat16

N_CORES = 8
B = 64
T = 31
E = 512
H = 1024
V = 32000
VK = V // N_CORES          # 4000 vocab cols per core
NN = 8                     # vocab n-tiles per core
NW = VK // NN              # 500
GK = 512                   # gate cols per core (f|i|o|g x 128)
R = T * B                  # 1984 output rows
NCHUNK = (T + 1) // 2      # 16 row chunks of 128 (last is 64)
NGRP = 4                   # proj n-tiles per group (one group per step window)


def _emit(tc):
    nc = tc.nc

    # ---------------- DRAM parameters ----------------
    embT = nc.dram_tensor("embT", [E, R], BF16, kind="ExternalInput")
    ctxT = nc.dram_tensor("ctxT", [H + 1, B], F32R, kind="ExternalInput")
    wxT = nc.dram_tensor("wxT", [E, GK], BF16, kind="ExternalInput")
    wcT = nc.dram_tensor("wcT", [H + 1, GK], F32R, kind="ExternalInput")
    whT = nc.dram_tensor("whT", [H, GK], BF16, kind="ExternalInput")
    linT = nc.dram_tensor("linT", [H, VK], BF16, kind="ExternalInput")
    idtr = nc.dram_tensor("idtr", [B, B], F32, kind="ExternalInput")
    id2 = nc.dram_tensor("id2", [128, B], BF16, kind="ExternalInput")
    idstk = nc.dram_tensor("idstk", [B, 128], BF16, kind="ExternalInput")
    out = nc.dram_tensor("out", [R, VK], BF16, kind="ExternalOutput")

    import contextlib

    ctx_es = contextlib.ExitStack()
    const = ctx_es.enter_context(tc.tile_pool(name="const", bufs=1))
    hrecp = ctx_es.enter_context(tc.tile_pool(name="hrecp", bufs=3))
    hprojp = ctx_es.enter_context(tc.tile_pool(name="hprojp", bufs=4))
    actp = ctx_es.enter_context(tc.tile_pool(name="actp", bufs=2))
    stagep = ctx_es.enter_context(tc.tile_pool(name="stagep", bufs=3))
    pp = ctx_es.enter_context(tc.tile_pool(name="pp", bufs=4, space="PSUM"))
    pg = ctx_es.enter_context(tc.tile_pool(name="pg", bufs=2, space="PSUM"))
    pt = ctx_es.enter_context(tc.tile_pool(name="pt", bufs=2, space="PSUM"))
    dramp = ctx_es.enter_context(tc.tile_pool(name="dramp", bufs=4, space="DRAM"))

    # ---------------- collective warm-up ----------------
    warm_sb = actp.tile([128, B], BF16, tag="warm")
    nc.vector.memset(warm_sb, 0.0)
    warm_in = dramp.tile([128, B], BF16, tag="warm_in")
    nc.sync.dma_start(out=warm_in, in_=warm_sb)
    warm_out = dramp.tile([N_CORES * 128, B], BF16, tag="warm_out",
                          addr_space="Shared")
    nc.gpsimd.collective_compute(
        "AllGather",
        mybir.AluOpType.bypass,
        replica_groups=[list(range(N_CORES))],
        ins=[warm_in.opt()],
        outs=[warm_out.opt()],
    )

    # ---------------- constant loads ----------------
    idtr_sb = const.tile([B, B], F32)
    nc.sync.dma_start(out=idtr_sb, in_=idtr[:, :])
    id2_sb = const.tile([128, B], BF16)
    nc.sync.dma_start(out=id2_sb, in_=id2[:, :])
    idstk_sb = const.tile([B, 128], BF16)
    nc.sync.dma_start(out=idstk_sb, in_=idstk[:, :])

    ctxm_sb = const.tile([128, 8 * B], F32R)
    for j in range(8):
        nc.sync.dma_start(out=ctxm_sb[:, j * B:(j + 1) * B], in_=ctxT[j * 128:(j + 1) * 128, :])
    ctxl_sb = const.tile([1, B], F32R)
    nc.sync.dma_start(out=ctxl_sb, in_=ctxT[H:H + 1, :])
    wcm_sb = const.tile([128, 8 * GK], F32R)
    for j in range(8):
        nc.sync.dma_start(out=wcm_sb[:, j * GK:(j + 1) * GK], in_=wcT[j * 128:(j + 1) * 128, :])
    wcl_sb = const.tile([1, GK], F32R)
    nc.sync.dma_start(out=wcl_sb, in_=wcT[H:H + 1, :])

    # WxT: 4 k-tiles [128, 512] bf16
    wx_sb = const.tile([128, 4 * GK], BF16)
    for e in range(4):
        nc.sync.dma_start(out=wx_sb[:, e * GK:(e + 1) * GK], in_=wxT[e * 128:(e + 1) * 128, :])
    # embT: 4 k-tiles of [128, 1984] bf16, split column-wise so early columns
    # (early timesteps) land first.
    embt_sb = const.tile([128, 4 * R], BF16)
    col_splits = [0, 256, 768, R]
    for ci in range(len(col_splits) - 1):
        c0, c1 = col_splits[ci], col_splits[ci + 1]
        for e in range(4):
            nc.sync.dma_start(
                out=embt_sb[:, e * R + c0:e * R + c1],
                in_=embT[e * 128:(e + 1) * 128, c0:c1],
            )
    # WhT: 8 k-tiles [128, 512]
    wh_sb = const.tile([128, 8 * GK], BF16)
    for j in range(8):
        nc.sync.dma_start(out=wh_sb[:, j * GK:(j + 1) * GK], in_=whT[j * 128:(j + 1) * 128, :])
    # linT: 8 k-tiles [128, 4000] bf16, halves, on the scalar HWDGE queue
    linw_sb = const.tile([128, 8 * VK], BF16)
    for j in range(8):
        for hhalf in range(2):
            c0 = hhalf * (VK // 2)
            nc.scalar.dma_start(
                out=linw_sb[:, j * VK + c0:j * VK + c0 + VK // 2],
                in_=linT[j * 128:(j + 1) * 128, c0:c0 + VK // 2],
            )

    # ---------------- cb = ctx @ Wc.T + bias  (64, 512) ----------------
    cb_ps = pg.tile([B, GK], F32, tag="pg")
    for j in range(8):
        nc.tensor.matmul(
            cb_ps, ctxm_sb[:, j * B:(j + 1) * B], wcm_sb[:, j * GK:(j + 1) * GK],
            start=(j == 0), stop=False,
        )
    nc.tensor.matmul(cb_ps, ctxl_sb, wcl_sb, start=False, stop=True)
    cb_sb = const.tile([B, GK], BF16)
    nc.scalar.copy(out=cb_sb, in_=cb_ps)

    # ---------------- x_pre (16 chunks of [128, 512], bf16) ----------------
    xpre_sb = const.tile([128, NCHUNK * GK], BF16)

    def emit_xpre(m):
        rows = 128 if m < NCHUNK - 1 else B
        xp = pp.tile([128, GK], F32, tag="pp")
        for e in range(4):
            nc.tensor.matmul(
                xp[:rows, :],
                embt_sb[:, e * R + m * 128:e * R + m * 128 + rows],
                wx_sb[:, e * GK:(e + 1) * GK],
                start=(e == 0), stop=False,
            )
        nc.tensor.matmul(xp[:rows, :], idstk_sb[:, :rows], cb_sb, start=False, stop=True)
        nc.scalar.copy(out=xpre_sb[:rows, m * GK:(m + 1) * GK], in_=xp[:rows, :])

    XPRE_AHEAD = 4
    for m in range(XPRE_AHEAD):
        emit_xpre(m)
    xpre_next = XPRE_AHEAD

    # ---------------- recurrence + projection ----------------
    c_prev = actp.tile([B, 128], F32, tag="c")
    nc.vector.memset(c_prev, 0.0)

    hrec_cur = None            # [128, 8*B] bf16: gathered hT (64 cols/shard)
    hproj_tiles = {}           # chunk -> [128, 8*128] bf16
    proj_tasks = []            # (chunk, ngrp) pending projection groups
    view = lambda ap, j, c0, w: ap[:, j * 128 + c0: j * 128 + c0 + w]

    stage_state = {}           # (chunk, grp) -> [tile, count]

    def emit_ntask(p_, n):
        rows = 128 if p_ < NCHUNK - 1 else B
        hp = hproj_tiles[p_]
        ps = pp.tile([128, GK], F32, tag="pp", name="ps")
        for j in range(8):
            nc.tensor.matmul(
                ps[:rows, :NW],
                view(hp, j, 0, rows),
                linw_sb[:, j * VK + n * NW:j * VK + (n + 1) * NW],
                start=(j == 0), stop=(j == 7),
            )
        grp = (p_, n // NGRP)
        if grp not in stage_state:
            stage_state[grp] = [stagep.tile([128, NGRP * NW], BF16, tag="st", name="st"), 0]
        st, cnt = stage_state[grp]
        ni = n % NGRP
        nc.scalar.copy(out=st[:rows, ni * NW:(ni + 1) * NW], in_=ps[:rows, :NW])
        stage_state[grp][1] = cnt + 1
        if cnt + 1 == NGRP:
            n0 = (n // NGRP) * NGRP * NW
            nc.gpsimd.dma_start(
                out=out[p_ * 128:p_ * 128 + rows, n0:n0 + NGRP * NW],
                in_=st[:rows, :NGRP * NW],
            )
            del stage_state[grp]

    for t in range(T):
        p, off = t // 2, B * (t & 1)
        # -- gates MM: 8 accumulating k-tile matmuls over the gathered h,
        # plus the precomputed x_pre term.
        gp = pg.tile([B, GK], F32, tag="pg")
        if t == 0:
            nc.tensor.matmul(
                gp, id2_sb[0:B, :B], xpre_sb[0:B, 0:GK], start=True, stop=True,
            )
        else:
            for j in range(8):
                nc.tensor.matmul(
                    gp, hrec_cur[:, j * B:(j + 1) * B],
                    wh_sb[:, j * GK:(j + 1) * GK],
                    start=(j == 0), stop=False,
                )
            nc.tensor.matmul(
                gp, id2_sb[off:off + B, :B],
                xpre_sb[off:off + B, p * GK:(p + 1) * GK],
                start=False, stop=True,
            )
        # -- activations: gates cols [f(0:128) i(128:256) o(256:384) g(384:512)]
        sfio = actp.tile([B, 384], F32, tag="sfio")
        nc.scalar.activation(out=sfio, in_=gp[:, 0:384], func=mybir.ActivationFunctionType.Sigmoid)
        t2 = actp.tile([B, 128], F32, tag="t2")
        nc.vector.tensor_mul(out=t2, in0=sfio[:, 0:128], in1=c_prev)
        gt = actp.tile([B, 128], F32, tag="gt")
        nc.scalar.activation(out=gt, in_=gp[:, 384:512], func=mybir.ActivationFunctionType.Tanh)
        t1 = actp.tile([B, 128], F32, tag="t1")
        nc.vector.tensor_mul(out=t1, in0=sfio[:, 128:256], in1=gt)
        c_new = actp.tile([B, 128], F32, tag="c")
        nc.vector.tensor_add(out=c_new, in0=t1, in1=t2)
        tc_ = actp.tile([B, 128], F32, tag="tc")
        nc.scalar.activation(out=tc_, in_=c_new, func=mybir.ActivationFunctionType.Tanh)
        h = actp.tile([B, 128], F32, tag="h")
        nc.vector.tensor_mul(out=h, in0=sfio[:, 256:384], in1=tc_)
        c_prev = c_new

        # -- transpose h on the PE, cast to bf16 (vector), send, AllGather
        ptr = pt.tile([128, B], F32, tag="pt")
        nc.tensor.transpose(ptr, h, idtr_sb)
        hts = actp.tile([128, B], BF16, tag="hts")
        nc.vector.tensor_copy(out=hts, in_=ptr)
        cc_in = dramp.tile([128, B], BF16, tag="cc_in")
        nc.scalar.dma_start(out=cc_in[:, :], in_=hts)
        cc_out = dramp.tile([N_CORES * 128, B], BF16, tag="cc_out",
                            addr_space="Shared")
        nc.gpsimd.collective_compute(
            "AllGather",
            mybir.AluOpType.bypass,
            replica_groups=[list(range(N_CORES))],
            ins=[cc_in.opt()],
            outs=[cc_out.opt()],
        )

        # -- receive: 2 strided DMAs on separate queues, 4 shards each, so
        # the first gates matmuls can start while the second half streams.
        hrec_next = hrecp.tile([128, 8 * B], BF16)
        scatter_eng = [nc.sync, nc.scalar]
        for j2 in range(2):
            scatter_eng[j2].dma_start(
                out=hrec_next[:, 4 * j2 * B:(4 * j2 + 4) * B].rearrange(
                    "q (j c) -> q j c", j=4),
                in_=cc_out[4 * j2 * 128:(4 * j2 + 4) * 128, :].rearrange(
                    "(j q) c -> q j c", j=4),
            )
        hrec_cur = hrec_next

        # -- copy gathered h into the chunk's projection-layout tile (vector)
        if p not in hproj_tiles:
            hproj_tiles[p] = hprojp.tile([128, 8 * 128], BF16, tag="hproj", name="hproj")
        hp = hproj_tiles[p]
        nc.vector.tensor_copy(
            out=hp[:, :].rearrange("q (j c) -> q j c", j=8)[:, :, off:off + B],
            in_=hrec_next[:, :].rearrange("q (j c) -> q j c", j=8),
        )

        # -- x_pre filler keeps PE busy during the early AllGather windows
        for _ in range(2):
            if xpre_next < NCHUNK and xpre_next <= 2 * t + 5:
                emit_xpre(xpre_next)
                xpre_next += 1

        # -- projection stream: up to 5 vocab n-tile tasks per window
        for _ in range(5):
            if proj_tasks:
                emit_ntask(*proj_tasks.pop(0))

        # -- PE keep-alive after the pops
        if t >= 3:
            kc = (t - 2) // 2
            hk = hproj_tiles[kc]
            ks = pg.tile([128, GK], F32, tag="pg", name="ka")
            for ki in range(2):
                nc.tensor.matmul(
                    ks, hk[:, 0:128], linw_sb[:, ki * GK:(ki + 1) * GK],
                    start=(ki == 0), stop=(ki == 1),
                )

        if (t & 1) or t == T - 1:
            for n in range(NN):
                proj_tasks.append((p, n))

    while proj_tasks:
        emit_ntask(*proj_tasks.pop(0))

    ctx_es.close()


_NC_CACHE = None


def _build():
    global _NC_CACHE
    if _NC_CACHE is None:
        nc = bacc.Bacc("TRN2", target_bir_lowering=False, debug=False,
                       num_devices=N_CORES)
        with tile.TileContext(nc) as tc:
            _emit(tc)
        nc.compile()
        _NC_CACHE = nc
    return _NC_CACHE


def kernel(features, captions, lengths, embed_table, W_ih, W_hh, b_ih, b_hh,
           attn_W, attn_b, lin_W, lin_b):
    f32 = np.float32
    bf16 = ml_dtypes.bfloat16
    features = np.asarray(features, f32)
    embed_table = np.asarray(embed_table, f32)
    W_ih = np.asarray(W_ih, f32)
    W_hh = np.asarray(W_hh, f32)
    b_ih = np.asarray(b_ih, f32)
    b_hh = np.asarray(b_hh, f32)
    lin_W = np.asarray(lin_W, f32)
    lin_b = np.asarray(lin_b, f32)
    cap = np.asarray(captions).astype(np.int64)[:, :T]

    # attention weights are softmax over a singleton dim == all ones
    ctx = features.sum(axis=1, dtype=f32)                      # (64, 1024)
    emb = embed_table[cap]                                     # (64, 31, 512)
    embT_np = np.ascontiguousarray(emb.transpose(2, 1, 0).reshape(E, R)).astype(bf16)
    ctxT_np = np.concatenate([ctx.T, np.ones((1, B), f32)], axis=0)  # (1025, 64)

    Wx = W_ih[:, :E]
    Wc = W_ih[:, E:]
    bias = (b_ih + b_hh).astype(f32)

    id64 = np.eye(B, dtype=f32)
    idtr_np = id64
    id2_np = np.concatenate([id64, id64], axis=0).astype(bf16)  # (128, 64)
    idstk_np = np.concatenate([id64, id64], axis=1).astype(bf16)  # (64, 128)

    in_maps = []
    for k in range(N_CORES):
        gidx = np.concatenate(
            [np.arange(k * 128, (k + 1) * 128) + o for o in (H, 0, 3 * H, 2 * H)]
        )  # [f i o g] rows for this core's 128 hidden dims
        vs = slice(k * VK, (k + 1) * VK)
        in_maps.append({
            "embT": embT_np,
            "ctxT": ctxT_np,
            "wxT": np.ascontiguousarray(Wx[gidx, :].T).astype(bf16),
            "wcT": np.ascontiguousarray(
                np.concatenate([Wc[gidx, :].T, bias[gidx][None, :]], axis=0), f32),
            "whT": np.ascontiguousarray(W_hh[gidx, :].T).astype(bf16),
            "linT": np.ascontiguousarray(lin_W[vs, :].T).astype(bf16),
            "idtr": idtr_np,
            "id2": id2_np,
            "idstk": idstk_np,
        })

    nc = _build()
    trace = bool(os.environ.get("ADR_TRACE"))
    kw = {}
    if trace:
        tmpdir = os.environ.get("ADR_TRACE_DIR") or None
        kw = dict(trace=True, tmpdir=tmpdir)
    res = run_bass_kernel_spmd(nc, in_maps, core_ids=list(range(N_CORES)), **kw)
    if trace:
        print(f"HW exec time: {res.exec_time_ns} ns", flush=True)

    out_full = np.concatenate(
        [np.asarray(res.results[k]["out"], dtype=f32) for k in range(N_CORES)], axis=1)
    out_full += lin_b[None, :]
    return out_full.astype(np.float32)


# revision 14
# speedup vs baseline: 1.0843x; 1.0843x over previous
"""AttentionDecoderRNN Trainium2 kernel (8 NeuronCores, SPMD).

Math (the reference's attention softmax is over a singleton dim -> weights all
ones -> ctx = features.sum(axis=1), constant over time):

    ctx   = features.sum(1)                                   (64, 1024)
    x_t   = embed[captions[:, t]]                             t = 0..30
    gates = [x_t, ctx] @ W_ih.T + b_ih + h @ W_hh.T + b_hh    (64, 4096)
    i,f,g,o -> LSTM update -> h_t                             (64, 1024)
    out[t*64+b, :] = h_t[b] @ lin_W.T + lin_b                 (1984, 32000)

Sharding: the LSTM recurrence is gate-sharded 8 ways (each core owns 128
hidden dims = 512 of the 4096 gate rows, reordered host-side to [f i o g]);
per step each core's h shard (64x128, bf16) is exchanged with an 8-rank
AllGather (Shared-addr-space output - the fast HBM-HBM path) and the
gathered (1024,64) buffer is brought back with two strided DMAs on separate
queues so the gates matmuls can start as soon as the first half lands.
The 32000-vocab output projection is column-sharded (4000 per core), runs in
bf16 on TensorE, and is paced one group per step. The x_pre precompute
(embeddings @ W_x + ctx term) runs in bf16. A dummy AllGather at kernel
start absorbs the collective entry barrier while weights stream in.
"""

import os
import sys

sys.path.insert(0, "/opt/trn_rl_repo")

import numpy as np
import ml_dtypes

import concourse.bass as bass
import concourse.tile as tile
from concourse.tile_rust import add_dep_helper
from concourse import bacc, mybir
from concourse.bass_utils import run_bass_kernel_spmd

F32 = mybir.dt.float32
F32R = mybir.dt.float32r
BF16 = mybir.dt.bfloat16

N_CORES = 8
B = 64
T = 31
E = 512
H = 1024
V = 32000
VK = V // N_CORES          # 4000 vocab cols per core
NN = 8                     # vocab n-tiles per core
NW = VK // NN              # 500
GK = 512                   # gate cols per core (f|i|o|g x 128)
R = T * B                  # 1984 output rows
NCHUNK = (T + 1) // 2      # 16 row chunks of 128 (last is 64)
NGRP = 4                   # proj n-tiles per group (one group per step window)


def _emit(tc):
    nc = tc.nc

    # ---------------- DRAM parameters ----------------
    embT = nc.dram_tensor("embT", [E, R], BF16, kind="ExternalInput")
    ctxT = nc.dram_tensor("ctxT", [H + 1, B], F32R, kind="ExternalInput")
    wxT = nc.dram_tensor("wxT", [E, GK], BF16, kind="ExternalInput")
    wcT = nc.dram_tensor("wcT", [H + 1, GK], F32R, kind="ExternalInput")
    whT = nc.dram_tensor("whT", [H, GK], BF16, kind="ExternalInput")
    linT = nc.dram_tensor("linT", [H, VK], BF16, kind="ExternalInput")
    idtr = nc.dram_tensor("idtr", [B, B], F32, kind="ExternalInput")
    id2 = nc.dram_tensor("id2", [128, B], BF16, kind="ExternalInput")
    idstk = nc.dram_tensor("idstk", [B, 128], BF16, kind="ExternalInput")
    out = nc.dram_tensor("out", [R, VK], BF16, kind="ExternalOutput")

    import contextlib

    ctx_es = contextlib.ExitStack()
    const = ctx_es.enter_context(tc.tile_pool(name="const", bufs=1))
    hrecp = ctx_es.enter_context(tc.tile_pool(name="hrecp", bufs=3))
    hprojp = ctx_es.enter_context(tc.tile_pool(name="hprojp", bufs=4))
    actp = ctx_es.enter_context(tc.tile_pool(name="actp", bufs=2))
    stagep = ctx_es.enter_context(tc.tile_pool(name="stagep", bufs=3))
    pp = ctx_es.enter_context(tc.tile_pool(name="pp", bufs=4, space="PSUM"))
    pg = ctx_es.enter_context(tc.tile_pool(name="pg", bufs=2, space="PSUM"))
    pt = ctx_es.enter_context(tc.tile_pool(name="pt", bufs=1, space="PSUM"))
    dramp = ctx_es.enter_context(tc.tile_pool(name="dramp", bufs=4, space="DRAM"))

    # ---------------- collective warm-up ----------------
    warm_sb = actp.tile([128, B], BF16, tag="warm")
    nc.vector.memset(warm_sb, 0.0)
    warm_in = dramp.tile([128, B], BF16, tag="warm_in")
    nc.sync.dma_start(out=warm_in, in_=warm_sb)
    warm_out = dramp.tile([N_CORES * 128, B], BF16, tag="warm_out",
                          addr_space="Shared")
    nc.gpsimd.collective_compute(
        "AllGather",
        mybir.AluOpType.bypass,
        replica_groups=[list(range(N_CORES))],
        ins=[warm_in.opt()],
        outs=[warm_out.opt()],
    )

    # ---------------- constant loads ----------------
    idtr_sb = const.tile([B, B], F32)
    nc.sync.dma_start(out=idtr_sb, in_=idtr[:, :])
    id2_sb = const.tile([128, B], BF16)
    nc.sync.dma_start(out=id2_sb, in_=id2[:, :])
    idstk_sb = const.tile([B, 128], BF16)
    nc.sync.dma_start(out=idstk_sb, in_=idstk[:, :])

    ctxm_sb = const.tile([128, 8 * B], F32R)
    for j in range(8):
        nc.sync.dma_start(out=ctxm_sb[:, j * B:(j + 1) * B], in_=ctxT[j * 128:(j + 1) * 128, :])
    ctxl_sb = const.tile([1, B], F32R)
    nc.sync.dma_start(out=ctxl_sb, in_=ctxT[H:H + 1, :])
    wcm_sb = const.tile([128, 8 * GK], F32R)
    for j in range(8):
        nc.sync.dma_start(out=wcm_sb[:, j * GK:(j + 1) * GK], in_=wcT[j * 128:(j + 1) * 128, :])
    wcl_sb = const.tile([1, GK], F32R)
    nc.sync.dma_start(out=wcl_sb, in_=wcT[H:H + 1, :])

    # WxT: 4 k-tiles [128, 512] bf16
    wx_sb = const.tile([128, 4 * GK], BF16)
    for e in range(4):
        nc.sync.dma_start(out=wx_sb[:, e * GK:(e + 1) * GK], in_=wxT[e * 128:(e + 1) * 128, :])
    # embT: 4 k-tiles of [128, 1984] bf16, split column-wise so early columns
    # (early timesteps) land first.
    embt_sb = const.tile([128, 4 * R], BF16)
    col_splits = [0, 256, 768, R]
    for ci in range(len(col_splits) - 1):
        c0, c1 = col_splits[ci], col_splits[ci + 1]
        for e in range(4):
            nc.sync.dma_start(
                out=embt_sb[:, e * R + c0:e * R + c1],
                in_=embT[e * 128:(e + 1) * 128, c0:c1],
            )
    # WhT: 8 k-tiles [128, 512]
    wh_sb = const.tile([128, 8 * GK], BF16)
    for j in range(8):
        nc.sync.dma_start(out=wh_sb[:, j * GK:(j + 1) * GK], in_=whT[j * 128:(j + 1) * 128, :])
    # linT: 8 k-tiles [128, 4000] bf16, halves, on the scalar HWDGE queue
    linw_sb = const.tile([128, 8 * VK], BF16)
    for j in range(8):
        for hhalf in range(2):
            c0 = hhalf * (VK // 2)
            nc.scalar.dma_start(
                out=linw_sb[:, j * VK + c0:j * VK + c0 + VK // 2],
                in_=linT[j * 128:(j + 1) * 128, c0:c0 + VK // 2],
            )

    # ---------------- cb = ctx @ Wc.T + bias  (64, 512) ----------------
    cb_ps = pg.tile([B, GK], F32, tag="pg", bufs=1)
    for j in range(8):
        nc.tensor.matmul(
            cb_ps, ctxm_sb[:, j * B:(j + 1) * B], wcm_sb[:, j * GK:(j + 1) * GK],
            start=(j == 0), stop=False,
        )
    nc.tensor.matmul(cb_ps, ctxl_sb, wcl_sb, start=False, stop=True)
    cb_sb = const.tile([B, GK], BF16)
    nc.scalar.copy(out=cb_sb, in_=cb_ps)

    # ---------------- x_pre (16 chunks of [128, 512], bf16) ----------------
    xpre_sb = const.tile([128, NCHUNK * GK], BF16)

    def emit_xpre(m):
        rows = 128 if m < NCHUNK - 1 else B
        xp = pp.tile([128, GK], F32, tag="pp")
        for e in range(4):
            mm = nc.tensor.matmul(
                xp[:rows, :],
                embt_sb[:, e * R + m * 128:e * R + m * 128 + rows],
                wx_sb[:, e * GK:(e + 1) * GK],
                start=(e == 0), stop=False,
            )
            if e == 0 and m >= 4:
                _pin(mm)
        nc.tensor.matmul(xp[:rows, :], idstk_sb[:, :rows], cb_sb, start=False, stop=True)
        nc.scalar.copy(out=xpre_sb[:rows, m * GK:(m + 1) * GK], in_=xp[:rows, :])

    XPRE_AHEAD = 4
    for m in range(XPRE_AHEAD):
        emit_xpre(m)
    xpre_next = XPRE_AHEAD

    # ---------------- recurrence + projection ----------------
    c_prev = actp.tile([B, 128], F32, tag="c")
    nc.vector.memset(c_prev, 0.0)

    pe_pin = [None, 0]         # [transpose-ins, free-slots-before-pin]
    def _pin(mm):
        # Order this PE task after the step's transpose unless it fits in
        # the act-chain window (free slots), so the transpose never queues
        # behind a long run of projection matmuls.
        if pe_pin[0] is not None:
            if pe_pin[1] > 0:
                pe_pin[1] -= 1
            else:
                add_dep_helper(mm.ins, pe_pin[0], sync=False,
                               reason="keep PE transpose ahead of proj")
        return mm

    hrec_cur = None            # [128, 8*B] bf16: gathered hT (64 cols/shard)
    hproj_tiles = {}           # chunk -> [128, 8*128] bf16
    proj_tasks = []            # (chunk, ngrp) pending projection groups
    view = lambda ap, j, c0, w: ap[:, j * 128 + c0: j * 128 + c0 + w]

    stage_state = {}           # (chunk, grp) -> [tile, count]

    def emit_ntask(p_, n):
        rows = 128 if p_ < NCHUNK - 1 else B
        hp = hproj_tiles[p_]
        ps = pp.tile([128, GK], F32, tag="pp", name="ps")
        for j in range(8):
            mm = nc.tensor.matmul(
                ps[:rows, :NW],
                view(hp, j, 0, rows),
                linw_sb[:, j * VK + n * NW:j * VK + (n + 1) * NW],
                start=(j == 0), stop=(j == 7),
            )
            if j == 0:
                _pin(mm)
        grp = (p_, n // NGRP)
        if grp not in stage_state:
            stage_state[grp] = [stagep.tile([128, NGRP * NW], BF16, tag="st", name="st"), 0]
        st, cnt = stage_state[grp]
        ni = n % NGRP
        nc.scalar.copy(out=st[:rows, ni * NW:(ni + 1) * NW], in_=ps[:rows, :NW])
        stage_state[grp][1] = cnt + 1
        if cnt + 1 == NGRP:
            n0 = (n // NGRP) * NGRP * NW
            nc.gpsimd.dma_start(
                out=out[p_ * 128:p_ * 128 + rows, n0:n0 + NGRP * NW],
                in_=st[:rows, :NGRP * NW],
            )
            del stage_state[grp]

    for t in range(T):
        p, off = t // 2, B * (t & 1)
        # -- gates MM: 8 accumulating k-tile matmuls over the gathered h,
        # plus the precomputed x_pre term.
        gpA = pg.tile([B, 384], F32, tag="pga", bufs=1)
        gpB = pg.tile([B, 128], F32, tag="pgb", bufs=1)
        if t == 0:
            nc.tensor.matmul(
                gpA, id2_sb[0:B, :B], xpre_sb[0:B, 0:384], start=True, stop=True,
            )
            nc.tensor.matmul(
                gpB, id2_sb[0:B, :B], xpre_sb[0:B, 384:GK], start=True, stop=True,
            )
        else:
            # A-group first (f,i,o): sigmoid can start while the g-group
            # matmuls still stream.
            for j in range(8):
                nc.tensor.matmul(
                    gpA, hrec_cur[:, j * B:(j + 1) * B],
                    wh_sb[:, j * GK:j * GK + 384],
                    start=(j == 0), stop=False,
                )
            nc.tensor.matmul(
                gpA, id2_sb[off:off + B, :B],
                xpre_sb[off:off + B, p * GK:p * GK + 384],
                start=False, stop=True,
            )
            for j in range(8):
                nc.tensor.matmul(
                    gpB, hrec_cur[:, j * B:(j + 1) * B],
                    wh_sb[:, j * GK + 384:(j + 1) * GK],
                    start=(j == 0), stop=False,
                )
            nc.tensor.matmul(
                gpB, id2_sb[off:off + B, :B],
                xpre_sb[off:off + B, p * GK + 384:(p + 1) * GK],
                start=False, stop=True,
            )
        # -- activations: gates cols [f(0:128) i(128:256) o(256:384) | g(0:128 of B)]
        sfio = actp.tile([B, 384], F32, tag="sfio")
        nc.scalar.activation(out=sfio, in_=gpA[:, 0:384], func=mybir.ActivationFunctionType.Sigmoid)
        t2 = actp.tile([B, 128], F32, tag="t2")
        nc.vector.tensor_mul(out=t2, in0=sfio[:, 0:128], in1=c_prev)
        gt = actp.tile([B, 128], F32, tag="gt")
        nc.scalar.activation(out=gt, in_=gpB[:, 0:128], func=mybir.ActivationFunctionType.Tanh)
        t1 = actp.tile([B, 128], F32, tag="t1")
        nc.vector.tensor_mul(out=t1, in0=sfio[:, 128:256], in1=gt)
        c_new = actp.tile([B, 128], F32, tag="c")
        nc.vector.tensor_add(out=c_new, in0=t1, in1=t2)
        tc_ = actp.tile([B, 128], F32, tag="tc")
        nc.scalar.activation(out=tc_, in_=c_new, func=mybir.ActivationFunctionType.Tanh)
        h = actp.tile([B, 128], F32, tag="h")
        nc.vector.tensor_mul(out=h, in0=sfio[:, 256:384], in1=tc_)
        c_prev = c_new

        # -- transpose h on the PE, cast to bf16 (vector), send, AllGather
        ptr = pt.tile([128, B], F32, tag="pt")
        tr_i = nc.tensor.transpose(ptr, h, idtr_sb)
        pe_pin[0] = tr_i.ins
        pe_pin[1] = 1
        hts = actp.tile([128, B], BF16, tag="hts")
        nc.vector.tensor_copy(out=hts, in_=ptr)
        cc_in = dramp.tile([128, B], BF16, tag="cc_in")
        nc.scalar.dma_start(out=cc_in[:, :], in_=hts)
        cc_out = dramp.tile([N_CORES * 128, B], BF16, tag="cc_out",
                            addr_space="Shared")
        nc.gpsimd.collective_compute(
            "AllGather",
            mybir.AluOpType.bypass,
            replica_groups=[list(range(N_CORES))],
            ins=[cc_in.opt()],
            outs=[cc_out.opt()],
        )

        # -- receive: 2 strided DMAs on separate queues, 4 shards each, so
        # the first gates matmuls can start while the second half streams.
        hrec_next = hrecp.tile([128, 8 * B], BF16)
        scatter_eng = [nc.sync, nc.scalar]
        for j in range(8):
            scatter_eng[j % 2].dma_start(
                out=hrec_next[:, j * B:(j + 1) * B],
                in_=cc_out[j * 128:(j + 1) * 128, :],
            )
        hrec_cur = hrec_next

        # -- copy gathered h into the chunk's projection-layout tile (vector)
        if p not in hproj_tiles:
            hproj_tiles[p] = hprojp.tile([128, 8 * 128], BF16, tag="hproj", name="hproj")
        hp = hproj_tiles[p]
        nc.vector.tensor_copy(
            out=hp[:, :].rearrange("q (j c) -> q j c", j=8)[:, :, off:off + B],
            in_=hrec_next[:, :].rearrange("q (j c) -> q j c", j=8),
        )

        # -- x_pre filler keeps PE busy during the early AllGather windows
        for _ in range(2):
            if xpre_next < NCHUNK and xpre_next <= 2 * t + 5:
                emit_xpre(xpre_next)
                xpre_next += 1

        # -- projection stream: up to 5 vocab n-tile tasks per window
        for _ in range(5):
            if proj_tasks:
                emit_ntask(*proj_tasks.pop(0))

        # -- PE keep-alive after the pops
        if t >= 3:
            kc = (t - 2) // 2
            hk = hproj_tiles[kc]
            ks = pp.tile([128, GK], F32, tag="pp", name="ka")
            for ki in range(2):
                mm = nc.tensor.matmul(
                    ks, hk[:, 0:128], linw_sb[:, ki * GK:(ki + 1) * GK],
                    start=(ki == 0), stop=(ki == 1),
                )
                if ki == 0:
                    _pin(mm)

        if (t & 1) or t == T - 1:
            for n in range(NN):
                proj_tasks.append((p, n))

    while proj_tasks:
        emit_ntask(*proj_tasks.pop(0))

    ctx_es.close()


_NC_CACHE = None


def _build():
    global _NC_CACHE
    if _NC_CACHE is None:
        nc = bacc.Bacc("TRN2", target_bir_lowering=False, debug=False,
                       num_devices=N_CORES)
        with tile.TileContext(nc) as tc:
            _emit(tc)
        nc.compile()
        _NC_CACHE = nc
    return _NC_CACHE


def kernel(features, captions, lengths, embed_table, W_ih, W_hh, b_ih, b_hh,
           attn_W, attn_b, lin_W, lin_b):
    f32 = np.float32
    bf16 = ml_dtypes.bfloat16
    features = np.asarray(features, f32)
    embed_table = np.asarray(embed_table, f32)
    W_ih = np.asarray(W_ih, f32)
    W_hh = np.asarray(W_hh, f32)
    b_ih = np.asarray(b_ih, f32)
    b_hh = np.asarray(b_hh, f32)
    lin_W = np.asarray(lin_W, f32)
    lin_b = np.asarray(lin_b, f32)
    cap = np.asarray(captions).astype(np.int64)[:, :T]

    # attention weights are softmax over a singleton dim == all ones
    ctx = features.sum(axis=1, dtype=f32)                      # (64, 1024)
    emb = embed_table[cap]                                     # (64, 31, 512)
    embT_np = np.ascontiguousarray(emb.transpose(2, 1, 0).reshape(E, R)).astype(bf16)
    ctxT_np = np.concatenate([ctx.T, np.ones((1, B), f32)], axis=0)  # (1025, 64)

    Wx = W_ih[:, :E]
    Wc = W_ih[:, E:]
    bias = (b_ih + b_hh).astype(f32)

    id64 = np.eye(B, dtype=f32)
    idtr_np = id64
    id2_np = np.concatenate([id64, id64], axis=0).astype(bf16)  # (128, 64)
    idstk_np = np.concatenate([id64, id64], axis=1).astype(bf16)  # (64, 128)

    in_maps = []
    for k in range(N_CORES):
        gidx = np.concatenate(
            [np.arange(k * 128, (k + 1) * 128) + o for o in (H, 0, 3 * H, 2 * H)]
        )  # [f i o g] rows for this core's 128 hidden dims
        vs = slice(k * VK, (k + 1) * VK)
        in_maps.append({
            "embT": embT_np,
            "ctxT": ctxT_np,
            "wxT": np.ascontiguousarray(Wx[gidx, :].T).astype(bf16),
            "wcT": np.ascontiguousarray(
                np.concatenate([Wc[gidx, :].T, bias[gidx][None, :]], axis=0), f32),
            "whT": np.ascontiguousarray(W_hh[gidx, :].T).astype(bf16),
            "linT": np.ascontiguousarray(lin_W[vs, :].T).astype(bf16),
            "idtr": idtr_np,
            "id2": id2_np,
            "idstk": idstk_np,
        })

    nc = _build()
    trace = bool(os.environ.get("ADR_TRACE"))
    kw = {}
    if trace:
        tmpdir = os.environ.get("ADR_TRACE_DIR") or None
        kw = dict(trace=True, tmpdir=tmpdir)
    res = run_bass_kernel_spmd(nc, in_maps, core_ids=list(range(N_CORES)), **kw)
    if trace:
        print(f"HW exec time: {res.exec_time_ns} ns", flush=True)

    out_full = np.concatenate(
        [np.asarray(res.results[k]["out"], dtype=f32) for k in range(N_CORES)], axis=1)
    out_full += lin_b[None, :]
    return out_full.astype(np.float32)
